# revision 35
# baseline (speedup 1.0000x reference)
"""Masked dot-product attention (B=64, L=1024, D=64, fp32) on 8 NeuronCores.

Strategy (data-parallel over batch, per the sharding hint):
  - Batches are sorted by valid_len (descending) and dealt round-robin to the
    8 cores so every core gets one batch from each of 8 "rank groups"; the
    per-slot key-block loop count is baked at build time as the max over that
    slot's rank group.  Key blocks that are entirely masked are never computed.
  - Scores are computed transposed, S^T[k, q] = K @ Q^T, via
    matmul(lhsT=K^T_slice, rhs=Q^T) so that the softmax axis (k) lands on the
    partition dim.  Q and K are passed pre-transposed [D, L] per batch (host
    layout choice at shard time; there is no 4-byte DMA transpose on TRN2).
  - The sequence mask is fused into the exp: ScalarE computes
    P^T = exp(S^T/8 + bias_k) with a per-partition bias column that is 0 for
    valid keys and -1e6 for masked keys (exp underflows to exactly 0).
    P^T is written in bf16.
  - AV runs TRANSPOSED: O[q, d] = P^T_chunk.T @ [V | 1], i.e.
    matmul(lhsT=P^T[:, qc*128:+128] (stationary), rhs=V_kb [128, 65] bf16
    (moving)).  Eight 65-row matmuls per key block replace two 512-row ones
    (bf16 runs 1 cycle/row at any size), and the output lands with q on the
    partition dim so the softmax denominator (ones column of V) is a
    per-partition scalar: the epilogue is one DVE reciprocal [128, 4] plus
    four tensor_scalar multiplies per half-slot -- no partition broadcast,
    no ScalarE copy, and the result is already in the host's [q, d] layout.
  - QK matmuls stay float32r (full PE rate at >=256 moving rows, ~1e-4 rel).

Scheduling notes (the in-order engine streams make emission order matter):
  - kb loop is software-pipelined: QK(kb+1) is emitted before AV(kb) so PE
    never parks behind an AV that waits on ScalarE's exp.
  - Pair/slot input DMAs are prefetched one slot ahead; the first pair's
    loads are split so the first QK only waits on ~320KB.
"""

import math
from contextlib import ExitStack

import numpy as np

import concourse.bass as bass
import concourse.bacc as bacc
import concourse.mybir as mybir
import concourse.tile as tile
from concourse.bass_utils import run_bass_kernel_spmd

F32 = mybir.dt.float32
F32R = mybir.dt.float32r
BF16 = mybir.dt.bfloat16
I16 = mybir.dt.int16
EXP = mybir.ActivationFunctionType.Exp

B, L, D = 64, 1024, 64
N_CORES = 8
SLOTS = B // N_CORES  # batches per core
KB = 128              # key-block size (partition dim of S^T)
N_KB = L // KB        # max key blocks
QH = 512              # q chunk per QK matmul (fp32 moving-operand max)
NQH = L // QH
NCH = L // KB         # q chunks of 128 for the transposed AV
NEG = -1000000.0

# Schraudolph fast-exp on DVE: i16 = round(x*log2e*128) + C, then bitcast
# int16 -> bf16 reads the integer as exponent|mantissa, i.e. 2^(i/128 - 127)
# with linear mantissa interpolation.  Summing two samples half an octave
# apart (i16 and i16+64) cancels most of the interpolation sawtooth:
# max rel err 1.43% (vs 3.3% single-sample) with C tuned numerically.
LOG2E = 1.4426950408889634
FEXP_A = 128.0 * LOG2E          # applied to x = s/sqrt(D) + bias
FEXP_C = 16086.5
FEXP_NEG = -80.0                # masked-key bias: keeps i16 positive, p~1e-35


def build_kernel(counts):
    """counts[s] = number of 128-wide key blocks to process for slot s."""
    nc = bacc.Bacc()

    qt_d = nc.dram_tensor("qt", [SLOTS, D, L], F32R, kind="ExternalInput")
    kt_d = nc.dram_tensor("kt", [SLOTS, D, L], F32R, kind="ExternalInput")
    v_d = nc.dram_tensor("v", [SLOTS, L, D + 1], BF16, kind="ExternalInput")
    bias_d = nc.dram_tensor("bias", [KB, SLOTS * N_KB], F32, kind="ExternalInput")
    bias16_d = nc.dram_tensor(
        "bias16", [KB, SLOTS * N_KB], F32, kind="ExternalInput"
    )
    out_d = nc.dram_tensor("out", [SLOTS, L, D], F32, kind="ExternalOutput")

    with tile.TileContext(nc) as tc, ExitStack() as ctx:
        const_pool = ctx.enter_context(tc.tile_pool(name="const", bufs=1))
        qk_pool = ctx.enter_context(tc.tile_pool(name="qk", bufs=3))
        v_pool = ctx.enter_context(tc.tile_pool(name="v", bufs=4))
        p_pool = ctx.enter_context(tc.tile_pool(name="p", bufs=6))
        fe_pool = ctx.enter_context(tc.tile_pool(name="fe", bufs=2))
        ep_pool = ctx.enter_context(tc.tile_pool(name="ep", bufs=4))
        out_pool = ctx.enter_context(tc.tile_pool(name="out", bufs=4))
        psum_s = ctx.enter_context(tc.tile_pool(name="psum_s", bufs=3, space="PSUM"))
        psum_o = ctx.enter_context(tc.tile_pool(name="psum_o", bufs=2, space="PSUM"))

        bias_t = const_pool.tile([KB, SLOTS * N_KB], F32)
        bias16_t = const_pool.tile([KB, SLOTS * N_KB], F32)
        warm_t = const_pool.tile([1, 1], F32)

        pair_tiles: dict[int, tuple] = {}
        v_tiles: dict[int, object] = {}
        pair_order = [1, 2, 3, 0]  # big pair last: tail epilogues hide in its long loops
        slot_order = [2 * p + h for p in pair_order for h in range(2)]
        next_pair = {pair_order[i]: pair_order[i + 1] for i in range(len(pair_order) - 1)}

        def load_pair(p):
            if p in pair_tiles:
                return
            n_max = counts[2 * p]
            # Two batches packed on the partition dim: even batch in
            # partitions 0-63, odd batch in 64-127.
            qt_t = qk_pool.tile([2 * D, L], F32R, tag="qt", name="qt_t")
            kt_t = qk_pool.tile([2 * D, L], F32R, tag="kt", name="kt_t")
            src_q = qt_d[2 * p : 2 * p + 2].rearrange("b d l -> (b d) l")
            src_k = kt_d[2 * p : 2 * p + 2].rearrange("b d l -> (b d) l")
            if not pair_tiles:
                # The first QKs' inputs ride both DGE queues in parallel
                # (SP and Activation are idle this early), smallest pieces
                # first so QK(0) unblocks ~3.6us in.
                c0 = min(2, n_max) * KB
                nc.sync.dma_start(qt_t[:D, QH:], src_q[:D, QH:])
                nc.scalar.dma_start(qt_t[:D, :QH], src_q[:D, :QH])
                nc.sync.dma_start(kt_t[:D, :KB], src_k[:D, :KB])
                if c0 > KB:
                    nc.sync.dma_start(kt_t[:D, KB:c0], src_k[:D, KB:c0])
                nc.sync.dma_start(kt_t[D:, :c0], src_k[D:, :c0])
                c1 = min(5, n_max) * KB
                if c1 > c0:
                    nc.sync.dma_start(kt_t[:, c0:c1], src_k[:, c0:c1])
                if n_max * KB > c1:
                    nc.sync.dma_start(
                        kt_t[:, c1 : n_max * KB], src_k[:, c1 : n_max * KB]
                    )
                nc.scalar.dma_start(qt_t[D:, :], src_q[D:, :])
            else:
                nc.sync.dma_start(qt_t[:], src_q)
                nc.sync.dma_start(kt_t[:, : n_max * KB], src_k[:, : n_max * KB])
            pair_tiles[p] = (qt_t, kt_t)

        def load_v(s):
            if s in v_tiles:
                return
            n_kb = counts[s]
            v_t = v_pool.tile([KB, N_KB, D + 1], BF16, name="v_t")
            nc.gpsimd.dma_start(
                v_t[:, :n_kb, :],
                v_d[s].rearrange("(n p) d -> p n d", p=KB)[:, :n_kb, :],
            )
            v_tiles[s] = v_t

        def qk(s_ps, rows, kt_t, qt_t, kb):
            for qh in range(NQH):
                nc.tensor.matmul(
                    s_ps[:, qh * QH : (qh + 1) * QH],
                    kt_t[rows, kb * KB : (kb + 1) * KB],
                    qt_t[rows, qh * QH : (qh + 1) * QH],
                    start=True,
                    stop=True,
                )

        load_pair(pair_order[0])
        # bias rides the SWDGE path so the first exp isn't queued behind
        # the HWDGE input loads.
        nc.gpsimd.dma_start(bias_t[:], bias_d[:])
        nc.gpsimd.dma_start(bias16_t[:], bias16_d[:])
        # Warm the exp table set while the first DMAs run.
        nc.scalar.activation(warm_t[:], bias_t[0:1, 0:1], EXP)
        load_v(slot_order[0])

        # PE p-state warm-up: keep the tensor engine continuously busy on
        # throwaway matmuls while the first input DMAs land, so the first
        # real QKs run at full clock instead of the cold 0.65GHz p-state.
        warm_sb = const_pool.tile([KB, QH], F32)
        nc.vector.memset(warm_sb[:], 0.0)
        for _ in range(5):
            w_ps = psum_o.tile([KB, 4 * KB], F32, tag="o", name="w_ps")
            nc.tensor.matmul(
                w_ps[:16, :],
                warm_sb[:, :16].bitcast(F32R),
                warm_sb[:].bitcast(F32R),
                start=True,
                stop=True,
            )

        # Flat (slot, kb) work list, software-pipelined at depth 2 across
        # slot boundaries: the PE stream is QK(i+1), AV(i-1), so PE never
        # refills the pipeline at a slot change and AV only ever consumes
        # an exp that finished a full iteration ago.
        work = [(s, kb) for s in slot_order for kb in range(counts[s])]
        n_work = len(work)
        o_tiles: dict[int, tuple] = {}
        s_tiles: dict[tuple, object] = {}
        p_tiles: dict[tuple, object] = {}

        def emit_qk(i):
            s, kb = work[i]
            pair, half = divmod(s, 2)
            if kb == 0:
                # Slot prologue: prefetch upcoming inputs.
                nxt = slot_order.index(s) + 1
                if nxt < SLOTS:
                    load_v(slot_order[nxt])
                    if nxt + 1 < SLOTS:
                        load_v(slot_order[nxt + 1])
                if half == 0 and pair in next_pair:
                    load_pair(next_pair[pair])
                if half == 1 and pair in next_pair and next_pair[pair] in next_pair:
                    load_pair(next_pair[next_pair[pair]])
            qt_t, kt_t = pair_tiles[pair]
            rows = slice(D * half, D * half + D)
            s_tiles[(s, kb)] = psum_s.tile([KB, L], F32, tag="s", name="s_ps")
            qk(s_tiles[(s, kb)], rows, kt_t, qt_t, kb)

        def emit_av(i):
            s, kb = work[i]
            n_kb = counts[s]
            if kb == 0:
                o_tiles[s] = (
                    psum_o.tile([KB, 4 * KB], F32, tag="o", name="o_ps_a"),
                    psum_o.tile([KB, 4 * KB], F32, tag="o", name="o_ps_b"),
                )
            o_a, o_b = o_tiles[s]
            p_ts = p_tiles.pop((s, kb))
            n_smp = len(p_ts)
            for smp, p_t in enumerate(p_ts):
                for ch in range(NCH):
                    g, cc = divmod(ch, 4)
                    o_g = o_a if g == 0 else o_b
                    p_ap = p_t[:, ch * KB : (ch + 1) * KB]
                    if p_ap.dtype == I16:
                        p_ap = p_ap.bitcast(BF16)
                    # PSUM start zeroes the whole 2KB bank (ZERO_REGION), so
                    # only the bank's first chunk starts the group and only
                    # its last chunk stops it; the middle chunks (and the
                    # second fast-exp sample, whose sum with the first is
                    # exactly the accumulation) add onto the zeros.
                    nc.tensor.matmul(
                        o_g[:, cc * KB : cc * KB + D + 1],
                        p_ap,
                        v_tiles[s][:, kb, :],
                        start=(kb == 0 and cc == 0 and smp == 0),
                        stop=(kb == n_kb - 1 and cc == 3 and smp == n_smp - 1),
                    )
            if kb == n_kb - 1:
                emit_epilogue(s)

        def emit_epilogue(s):
            # Per-partition denominators: one reciprocal [128, 4], one
            # tensor_tensor multiply against a stride-0 broadcast view of it,
            # one store per half-slot (already in [q, d] layout).
            recs = []
            for g in range(2):
                rec4 = ep_pool.tile([KB, 4], F32, name="rec4")
                nc.vector.reciprocal(rec4[:], o_tiles[s][g][:, D : 4 * KB : KB])
                recs.append(rec4)
            for g in range(2):
                o_g = o_tiles[s][g]
                rec4 = recs[g]
                out_sb = out_pool.tile([KB, 4, D], F32, name="out_sb")
                rap = rec4[:]
                rec_b = bass.AP(
                    rap.tensor, rap.offset, list(rap.ap) + [[0, D]]
                )
                src = bass.AP(
                    o_g.tensor,
                    o_g[:].offset,
                    [list(o_g[:].ap)[0], [KB, 4], [1, D]],
                )
                nc.vector.tensor_tensor(
                    out_sb[:], src, rec_b, op=mybir.AluOpType.mult
                )
                dst = out_d[s][g * 4 * KB : (g + 1) * 4 * KB].rearrange(
                    "(c p) d -> p c d", p=KB
                )
                nc.sync.dma_start(dst, out_sb[:])

        # Work items whose exp runs as a two-sample Schraudolph fast-exp on
        # DVE instead of ScalarE: evenly spaced, sparing the pipeline head
        # and the exposed tail.
        n_off = min(14, max(0, (n_work - 6) // 2))
        offload = set()
        if n_off:
            span = n_work - 6  # usable indices [3, n_work-4]
            offload = {3 + (k * span) // n_off for k in range(n_off)}

        AV_DEPTH = 3  # iterations between an exp and its AV consumption

        emit_qk(0)
        for i in range(n_work):
            if i + 1 < n_work:
                emit_qk(i + 1)
            # AV (and slot epilogues, whose DVE reciprocals must not queue
            # behind a later fast-exp) are emitted before this item's exp.
            # The depth-2 lag gives the DVE fast-exp chain two full
            # iterations to finish before PE blocks on its result.
            if i >= AV_DEPTH:
                emit_av(i - AV_DEPTH)
            s, kb = work[i]
            s_ps = s_tiles.pop((s, kb))
            col = s * N_KB + kb
            if i in offload:
                # Two-sample fast-exp: the AV matmuls consume BOTH samples
                # and the PSUM accumulation performs their sum for free.
                t1 = fe_pool.tile([KB, L], I16, tag="t1", name="t1")
                t2 = fe_pool.tile([KB, L], I16, tag="t2", name="t2")
                p_tiles[(s, kb)] = (t1, t2)
                nc.vector.tensor_scalar(
                    t1[:],
                    s_ps[:],
                    FEXP_A / math.sqrt(D),
                    bias16_t[:, col : col + 1],
                    op0=mybir.AluOpType.mult,
                    op1=mybir.AluOpType.add,
                )
                nc.vector.tensor_scalar(
                    t2[:], t1[:], 64, None, op0=mybir.AluOpType.add
                )
            else:
                p_t = p_pool.tile([KB, L], BF16, name="p_t")
                p_tiles[(s, kb)] = (p_t,)
                # The first and last exps are split in q-halves: the first
                # so ScalarE starts after only half of QK(0), the last so
                # the exposed tail epilogue chain starts earlier.
                halves = 2 if i in (0, n_work - 1) else 1
                hw_ = L // halves
                for h in range(halves):
                    nc.scalar.activation(
                        p_t[:, h * hw_ : (h + 1) * hw_],
                        s_ps[:, h * hw_ : (h + 1) * hw_],
                        EXP,
                        bias=bias_t[:, col : col + 1],
                        scale=1.0 / math.sqrt(D),
                    )
        for i in range(max(0, n_work - AV_DEPTH), n_work):
            emit_av(i)

    nc.finalize()
    return nc


_NC_CACHE: dict[tuple, object] = {}


def _prepare(queries, keys, values, valid_lens):
    queries = np.ascontiguousarray(queries, dtype=np.float32)
    keys = np.ascontiguousarray(keys, dtype=np.float32)
    values = np.ascontiguousarray(values, dtype=np.float32)
    valid_lens = np.asarray(valid_lens)
    assert queries.shape == (B, L, D), queries.shape
    vl = valid_lens.astype(np.int64)

    # Sort batches by valid_len descending; slot s on core c gets the batch
    # of rank s*8 + c.  Each slot's loop count covers the max valid_len in
    # its rank group, so one instruction stream fits all cores.
    order = np.argsort(-vl, kind="stable")
    counts = tuple(
        max(1, math.ceil(int(vl[order[s * N_CORES]]) / KB)) for s in range(SLOTS)
    )
    # Pairs share a K^T tile sized by the even slot; counts are descending.
    nc = _NC_CACHE.get(counts)
    if nc is None:
        nc = build_kernel(counts)
        _NC_CACHE[counts] = nc

    bf16 = mybir.dt.np(BF16)
    col = np.arange(L)
    in_maps = []
    for c in range(N_CORES):
        batch_idx = [int(order[s * N_CORES + c]) for s in range(SLOTS)]
        qt = np.ascontiguousarray(
            queries[batch_idx].transpose(0, 2, 1)
        )  # [SLOTS, D, L]
        kt = np.ascontiguousarray(keys[batch_idx].transpose(0, 2, 1))
        v = np.concatenate(
            [values[batch_idx], np.ones((SLOTS, L, 1), np.float32)], axis=2
        ).astype(bf16)
        bias = np.zeros((KB, SLOTS * N_KB), dtype=np.float32)
        for s in range(SLOTS):
            mask = (col >= vl[batch_idx[s]]).astype(np.float32) * NEG  # [L]
            bias[:, s * N_KB : (s + 1) * N_KB] = mask.reshape(N_KB, KB).T
        # Schraudolph constants for the DVE fast-exp blocks: the affine
        # instruction computes s*(FEXP_A/8) + bias16, so bias16 folds both
        # the masked-key offset and the exponent-bias magic constant.
        bias16 = np.where(bias < -1.0, FEXP_NEG * FEXP_A + FEXP_C, FEXP_C)
        bias16 = bias16.astype(np.float32)
        in_maps.append(
            {"qt": qt, "kt": kt, "v": v, "bias": bias, "bias16": bias16}
        )
    return nc, in_maps, order


def _unshard(res, order):
    out = np.empty((B, L, D), dtype=np.float32)
    for c in range(N_CORES):
        o = res.results[c]["out"]  # [SLOTS, L, D]
        for s in range(SLOTS):
            out[int(order[s * N_CORES + c])] = o[s]
    return out


def kernel(queries, keys, values, valid_lens):
    nc, in_maps, order = _prepare(queries, keys, values, valid_lens)
    res = run_bass_kernel_spmd(nc, in_maps, core_ids=list(range(N_CORES)))
    return _unshard(res, order)


def trace_run(queries, keys, values, valid_lens):
    """Like kernel() but traced; returns BassKernelResults (for test.py)."""
    nc, in_maps, order = _prepare(queries, keys, values, valid_lens)
    res = run_bass_kernel_spmd(
        nc, in_maps, core_ids=list(range(N_CORES)), trace=True
    )
    res.full_output = _unshard(res, order)
    return res


# revision 36
# speedup vs baseline: 1.0195x; 1.0195x over previous
"""Masked dot-product attention (B=64, L=1024, D=64, fp32) on 8 NeuronCores.

Strategy (data-parallel over batch, per the sharding hint):
  - Batches are sorted by valid_len (descending) and dealt round-robin to the
    8 cores so every core gets one batch from each of 8 "rank groups"; the
    per-slot key-block loop count is baked at build time as the max over that
    slot's rank group.  Key blocks that are entirely masked are never computed.
  - Scores are computed transposed, S^T[k, q] = K @ Q^T, via
    matmul(lhsT=K^T_slice, rhs=Q^T) so that the softmax axis (k) lands on the
    partition dim.  Q and K are passed pre-transposed [D, L] per batch (host
    layout choice at shard time; there is no 4-byte DMA transpose on TRN2).
  - The sequence mask is fused into the exp: ScalarE computes
    P^T = exp(S^T/8 + bias_k) with a per-partition bias column that is 0 for
    valid keys and -1e6 for masked keys (exp underflows to exactly 0).
    P^T is written in bf16.
  - AV runs TRANSPOSED: O[q, d] = P^T_chunk.T @ [V | 1], i.e.
    matmul(lhsT=P^T[:, qc*128:+128] (stationary), rhs=V_kb [128, 65] bf16
    (moving)).  Eight 65-row matmuls per key block replace two 512-row ones
    (bf16 runs 1 cycle/row at any size), and the output lands with q on the
    partition dim so the softmax denominator (ones column of V) is a
    per-partition scalar: the epilogue is one DVE reciprocal [128, 4] plus
    four tensor_scalar multiplies per half-slot -- no partition broadcast,
    no ScalarE copy, and the result is already in the host's [q, d] layout.
  - QK matmuls stay float32r (full PE rate at >=256 moving rows, ~1e-4 rel).

Scheduling notes (the in-order engine streams make emission order matter):
  - kb loop is software-pipelined: QK(kb+1) is emitted before AV(kb) so PE
    never parks behind an AV that waits on ScalarE's exp.
  - Pair/slot input DMAs are prefetched one slot ahead; the first pair's
    loads are split so the first QK only waits on ~320KB.
"""

import math
from contextlib import ExitStack

import numpy as np

import concourse.bass as bass
import concourse.bacc as bacc
import concourse.mybir as mybir
import concourse.tile as tile
from concourse.bass_utils import run_bass_kernel_spmd

F32 = mybir.dt.float32
F32R = mybir.dt.float32r
BF16 = mybir.dt.bfloat16
I16 = mybir.dt.int16
EXP = mybir.ActivationFunctionType.Exp

B, L, D = 64, 1024, 64
N_CORES = 8
SLOTS = B // N_CORES  # batches per core
KB = 128              # key-block size (partition dim of S^T)
N_KB = L // KB        # max key blocks
QH = 512              # q chunk per QK matmul (fp32 moving-operand max)
NQH = L // QH
NCH = L // KB         # q chunks of 128 for the transposed AV
NEG = -1000000.0

# Schraudolph fast-exp on DVE: i16 = round(x*log2e*128) + C, then bitcast
# int16 -> bf16 reads the integer as exponent|mantissa, i.e. 2^(i/128 - 127)
# with linear mantissa interpolation.  Summing two samples half an octave
# apart (i16 and i16+64) cancels most of the interpolation sawtooth:
# max rel err 1.43% (vs 3.3% single-sample) with C tuned numerically.
LOG2E = 1.4426950408889634
FEXP_A = 128.0 * LOG2E          # applied to x = s/sqrt(D) + bias
FEXP_C = 16086.5
FEXP_NEG = -80.0                # masked-key bias: keeps i16 positive, p~1e-35


def build_kernel(counts):
    """counts[s] = number of 128-wide key blocks to process for slot s."""
    nc = bacc.Bacc()

    qt_d = nc.dram_tensor("qt", [SLOTS, D, L], F32R, kind="ExternalInput")
    kt_d = nc.dram_tensor("kt", [SLOTS, D, L], F32R, kind="ExternalInput")
    v_d = nc.dram_tensor("v", [SLOTS, L, D + 1], BF16, kind="ExternalInput")
    bias_d = nc.dram_tensor("bias", [KB, SLOTS * N_KB], F32, kind="ExternalInput")
    bias16_d = nc.dram_tensor(
        "bias16", [KB, SLOTS * N_KB], F32, kind="ExternalInput"
    )
    out_d = nc.dram_tensor("out", [SLOTS, L, D], F32, kind="ExternalOutput")

    with tile.TileContext(nc) as tc, ExitStack() as ctx:
        const_pool = ctx.enter_context(tc.tile_pool(name="const", bufs=1))
        qk_pool = ctx.enter_context(tc.tile_pool(name="qk", bufs=3))
        v_pool = ctx.enter_context(tc.tile_pool(name="v", bufs=4))
        p_pool = ctx.enter_context(tc.tile_pool(name="p", bufs=6))
        fe_pool = ctx.enter_context(tc.tile_pool(name="fe", bufs=2))
        ep_pool = ctx.enter_context(tc.tile_pool(name="ep", bufs=4))
        out_pool = ctx.enter_context(tc.tile_pool(name="out", bufs=4))
        psum_s = ctx.enter_context(tc.tile_pool(name="psum_s", bufs=3, space="PSUM"))
        psum_o = ctx.enter_context(tc.tile_pool(name="psum_o", bufs=2, space="PSUM"))

        bias_t = const_pool.tile([KB, SLOTS * N_KB], F32)
        bias16_t = const_pool.tile([KB, SLOTS * N_KB], F32)
        warm_t = const_pool.tile([1, 1], F32)

        pair_tiles: dict[int, tuple] = {}
        v_tiles: dict[int, object] = {}
        pair_order = [1, 2, 3, 0]  # big pair last: tail epilogues hide in its long loops
        slot_order = [2 * p + h for p in pair_order for h in range(2)]
        next_pair = {pair_order[i]: pair_order[i + 1] for i in range(len(pair_order) - 1)}

        def load_pair(p):
            if p in pair_tiles:
                return
            n_max = counts[2 * p]
            # Two batches packed on the partition dim: even batch in
            # partitions 0-63, odd batch in 64-127.
            qt_t = qk_pool.tile([2 * D, L], F32R, tag="qt", name="qt_t")
            kt_t = qk_pool.tile([2 * D, L], F32R, tag="kt", name="kt_t")
            src_q = qt_d[2 * p : 2 * p + 2].rearrange("b d l -> (b d) l")
            src_k = kt_d[2 * p : 2 * p + 2].rearrange("b d l -> (b d) l")
            if not pair_tiles:
                # The first QKs' inputs ride both DGE queues in parallel
                # (SP and Activation are idle this early), smallest pieces
                # first so QK(0) unblocks ~3.6us in.
                c0 = min(2, n_max) * KB
                nc.sync.dma_start(qt_t[:D, QH:], src_q[:D, QH:])
                nc.scalar.dma_start(qt_t[:D, :QH], src_q[:D, :QH])
                nc.sync.dma_start(kt_t[:D, :KB], src_k[:D, :KB])
                if c0 > KB:
                    nc.sync.dma_start(kt_t[:D, KB:c0], src_k[:D, KB:c0])
                nc.sync.dma_start(kt_t[D:, :c0], src_k[D:, :c0])
                c1 = min(5, n_max) * KB
                if c1 > c0:
                    nc.sync.dma_start(kt_t[:, c0:c1], src_k[:, c0:c1])
                if n_max * KB > c1:
                    nc.sync.dma_start(
                        kt_t[:, c1 : n_max * KB], src_k[:, c1 : n_max * KB]
                    )
                nc.scalar.dma_start(qt_t[D:, :], src_q[D:, :])
            else:
                nc.sync.dma_start(qt_t[:], src_q)
                nc.sync.dma_start(kt_t[:, : n_max * KB], src_k[:, : n_max * KB])
            pair_tiles[p] = (qt_t, kt_t)

        def load_v(s):
            if s in v_tiles:
                return
            n_kb = counts[s]
            v_t = v_pool.tile([KB, N_KB, D + 1], BF16, name="v_t")
            nc.gpsimd.dma_start(
                v_t[:, :n_kb, :],
                v_d[s].rearrange("(n p) d -> p n d", p=KB)[:, :n_kb, :],
            )
            v_tiles[s] = v_t

        def qk(s_ps, rows, kt_t, qt_t, kb):
            for qh in range(NQH):
                nc.tensor.matmul(
                    s_ps[:, qh * QH : (qh + 1) * QH],
                    kt_t[rows, kb * KB : (kb + 1) * KB],
                    qt_t[rows, qh * QH : (qh + 1) * QH],
                    start=True,
                    stop=True,
                )

        load_pair(pair_order[0])
        # bias rides the SWDGE path so the first exp isn't queued behind
        # the HWDGE input loads.
        nc.gpsimd.dma_start(bias_t[:], bias_d[:])
        nc.gpsimd.dma_start(bias16_t[:], bias16_d[:])
        # Warm the exp table set while the first DMAs run.
        nc.scalar.activation(warm_t[:], bias_t[0:1, 0:1], EXP)
        load_v(slot_order[0])

        # PE p-state warm-up: keep the tensor engine continuously busy on
        # throwaway matmuls while the first input DMAs land, so the first
        # real QKs run at full clock instead of the cold 0.65GHz p-state.
        warm_sb = const_pool.tile([KB, QH], F32)
        nc.vector.memset(warm_sb[:], 0.0)
        for _ in range(5):
            w_ps = psum_o.tile([KB, 4 * KB], F32, tag="o", name="w_ps")
            nc.tensor.matmul(
                w_ps[:16, :],
                warm_sb[:, :16].bitcast(F32R),
                warm_sb[:].bitcast(F32R),
                start=True,
                stop=True,
            )

        # Flat (slot, kb) work list, software-pipelined at depth 2 across
        # slot boundaries: the PE stream is QK(i+1), AV(i-1), so PE never
        # refills the pipeline at a slot change and AV only ever consumes
        # an exp that finished a full iteration ago.
        work = [(s, kb) for s in slot_order for kb in range(counts[s])]
        n_work = len(work)
        o_tiles: dict[int, tuple] = {}
        s_tiles: dict[tuple, object] = {}
        p_tiles: dict[tuple, object] = {}

        def emit_qk(i):
            s, kb = work[i]
            pair, half = divmod(s, 2)
            if kb == 0:
                # Slot prologue: prefetch upcoming inputs.
                nxt = slot_order.index(s) + 1
                if nxt < SLOTS:
                    load_v(slot_order[nxt])
                    if nxt + 1 < SLOTS:
                        load_v(slot_order[nxt + 1])
                if half == 0 and pair in next_pair:
                    load_pair(next_pair[pair])
                if half == 1 and pair in next_pair and next_pair[pair] in next_pair:
                    load_pair(next_pair[next_pair[pair]])
            qt_t, kt_t = pair_tiles[pair]
            rows = slice(D * half, D * half + D)
            s_tiles[(s, kb)] = psum_s.tile([KB, L], F32, tag="s", name="s_ps")
            qk(s_tiles[(s, kb)], rows, kt_t, qt_t, kb)

        def emit_av(i):
            s, kb = work[i]
            n_kb = counts[s]
            if kb == 0:
                o_tiles[s] = (
                    psum_o.tile([KB, 4 * KB], F32, tag="o", name="o_ps_a"),
                    psum_o.tile([KB, 4 * KB], F32, tag="o", name="o_ps_b"),
                )
            o_a, o_b = o_tiles[s]
            p_ts = p_tiles.pop((s, kb))
            n_smp = len(p_ts)
            for smp, p_t in enumerate(p_ts):
                for ch in range(NCH):
                    g, cc = divmod(ch, 4)
                    o_g = o_a if g == 0 else o_b
                    p_ap = p_t[:, ch * KB : (ch + 1) * KB]
                    if p_ap.dtype == I16:
                        p_ap = p_ap.bitcast(BF16)
                    # PSUM start zeroes the whole 2KB bank (ZERO_REGION), so
                    # only the bank's first chunk starts the group and only
                    # its last chunk stops it; the middle chunks (and the
                    # second fast-exp sample, whose sum with the first is
                    # exactly the accumulation) add onto the zeros.
                    nc.tensor.matmul(
                        o_g[:, cc * KB : cc * KB + D + 1],
                        p_ap,
                        v_tiles[s][:, kb, :],
                        start=(kb == 0 and cc == 0 and smp == 0),
                        stop=(kb == n_kb - 1 and cc == 3 and smp == n_smp - 1),
                    )
            if kb == n_kb - 1:
                emit_epilogue(s)

        def emit_epilogue(s):
            # Per-partition denominators: one reciprocal [128, 4], one
            # tensor_tensor multiply against a stride-0 broadcast view of it,
            # one store per half-slot (already in [q, d] layout).
            recs = []
            for g in range(2):
                rec4 = ep_pool.tile([KB, 4], F32, name="rec4")
                nc.vector.reciprocal(rec4[:], o_tiles[s][g][:, D : 4 * KB : KB])
                recs.append(rec4)
            for g in range(2):
                o_g = o_tiles[s][g]
                rec4 = recs[g]
                out_sb = out_pool.tile([KB, 4, D], F32, name="out_sb")
                rap = rec4[:]
                rec_b = bass.AP(
                    rap.tensor, rap.offset, list(rap.ap) + [[0, D]]
                )
                src = bass.AP(
                    o_g.tensor,
                    o_g[:].offset,
                    [list(o_g[:].ap)[0], [KB, 4], [1, D]],
                )
                nc.vector.tensor_tensor(
                    out_sb[:], src, rec_b, op=mybir.AluOpType.mult
                )
                dst = out_d[s][g * 4 * KB : (g + 1) * 4 * KB].rearrange(
                    "(c p) d -> p c d", p=KB
                )
                nc.sync.dma_start(dst, out_sb[:])

        # Work items whose exp runs as a two-sample Schraudolph fast-exp on
        # DVE instead of ScalarE: evenly spaced, sparing the pipeline head
        # and the exposed tail.
        offload = {i for i in range(2, n_work - 2) if i % 3 == 1}

        AV_DEPTH = 3  # iterations between an exp and its AV consumption

        emit_qk(0)
        for i in range(n_work):
            if i + 1 < n_work:
                emit_qk(i + 1)
            # AV (and slot epilogues, whose DVE reciprocals must not queue
            # behind a later fast-exp) are emitted before this item's exp.
            # The depth-2 lag gives the DVE fast-exp chain two full
            # iterations to finish before PE blocks on its result.
            if i >= AV_DEPTH:
                emit_av(i - AV_DEPTH)
            s, kb = work[i]
            s_ps = s_tiles.pop((s, kb))
            col = s * N_KB + kb
            if i in offload:
                # Two-sample fast-exp: the AV matmuls consume BOTH samples
                # and the PSUM accumulation performs their sum for free.
                t1 = fe_pool.tile([KB, L], I16, tag="t1", name="t1")
                t2 = fe_pool.tile([KB, L], I16, tag="t2", name="t2")
                p_tiles[(s, kb)] = (t1, t2)
                nc.vector.tensor_scalar(
                    t1[:],
                    s_ps[:],
                    FEXP_A / math.sqrt(D),
                    bias16_t[:, col : col + 1],
                    op0=mybir.AluOpType.mult,
                    op1=mybir.AluOpType.add,
                )
                nc.vector.tensor_scalar(
                    t2[:], t1[:], 64, None, op0=mybir.AluOpType.add
                )
            else:
                p_t = p_pool.tile([KB, L], BF16, name="p_t")
                p_tiles[(s, kb)] = (p_t,)
                # The first and last exps are split in q-halves: the first
                # so ScalarE starts after only half of QK(0), the last so
                # the exposed tail epilogue chain starts earlier.
                halves = 2 if i in (0, n_work - 1) else 1
                hw_ = L // halves
                for h in range(halves):
                    nc.scalar.activation(
                        p_t[:, h * hw_ : (h + 1) * hw_],
                        s_ps[:, h * hw_ : (h + 1) * hw_],
                        EXP,
                        bias=bias_t[:, col : col + 1],
                        scale=1.0 / math.sqrt(D),
                    )
        for i in range(max(0, n_work - AV_DEPTH), n_work):
            emit_av(i)

    nc.finalize()
    return nc


_NC_CACHE: dict[tuple, object] = {}


def _prepare(queries, keys, values, valid_lens):
    queries = np.ascontiguousarray(queries, dtype=np.float32)
    keys = np.ascontiguousarray(keys, dtype=np.float32)
    values = np.ascontiguousarray(values, dtype=np.float32)
    valid_lens = np.asarray(valid_lens)
    assert queries.shape == (B, L, D), queries.shape
    vl = valid_lens.astype(np.int64)

    # Sort batches by valid_len descending; slot s on core c gets the batch
    # of rank s*8 + c.  Each slot's loop count covers the max valid_len in
    # its rank group, so one instruction stream fits all cores.
    order = np.argsort(-vl, kind="stable")
    counts = tuple(
        max(1, math.ceil(int(vl[order[s * N_CORES]]) / KB)) for s in range(SLOTS)
    )
    # Pairs share a K^T tile sized by the even slot; counts are descending.
    nc = _NC_CACHE.get(counts)
    if nc is None:
        nc = build_kernel(counts)
        _NC_CACHE[counts] = nc

    bf16 = mybir.dt.np(BF16)
    col = np.arange(L)
    in_maps = []
    for c in range(N_CORES):
        batch_idx = [int(order[s * N_CORES + c]) for s in range(SLOTS)]
        qt = np.ascontiguousarray(
            queries[batch_idx].transpose(0, 2, 1)
        )  # [SLOTS, D, L]
        kt = np.ascontiguousarray(keys[batch_idx].transpose(0, 2, 1))
        v = np.concatenate(
            [values[batch_idx], np.ones((SLOTS, L, 1), np.float32)], axis=2
        ).astype(bf16)
        bias = np.zeros((KB, SLOTS * N_KB), dtype=np.float32)
        for s in range(SLOTS):
            mask = (col >= vl[batch_idx[s]]).astype(np.float32) * NEG  # [L]
            bias[:, s * N_KB : (s + 1) * N_KB] = mask.reshape(N_KB, KB).T
        # Schraudolph constants for the DVE fast-exp blocks: the affine
        # instruction computes s*(FEXP_A/8) + bias16, so bias16 folds both
        # the masked-key offset and the exponent-bias magic constant.
        bias16 = np.where(bias < -1.0, FEXP_NEG * FEXP_A + FEXP_C, FEXP_C)
        bias16 = bias16.astype(np.float32)
        in_maps.append(
            {"qt": qt, "kt": kt, "v": v, "bias": bias, "bias16": bias16}
        )
    return nc, in_maps, order


def _unshard(res, order):
    out = np.empty((B, L, D), dtype=np.float32)
    for c in range(N_CORES):
        o = res.results[c]["out"]  # [SLOTS, L, D]
        for s in range(SLOTS):
            out[int(order[s * N_CORES + c])] = o[s]
    return out


def kernel(queries, keys, values, valid_lens):
    nc, in_maps, order = _prepare(queries, keys, values, valid_lens)
    res = run_bass_kernel_spmd(nc, in_maps, core_ids=list(range(N_CORES)))
    return _unshard(res, order)


def trace_run(queries, keys, values, valid_lens):
    """Like kernel() but traced; returns BassKernelResults (for test.py)."""
    nc, in_maps, order = _prepare(queries, keys, values, valid_lens)
    res = run_bass_kernel_spmd(
        nc, in_maps, core_ids=list(range(N_CORES)), trace=True
    )
    res.full_output = _unshard(res, order)
    return res
